# revision 26
# baseline (speedup 1.0000x reference)
"""DigitCaps dynamic-routing kernel for Trainium2, 8 NeuronCores (SPMD).

Problem:  in_caps [64, 2048, 16] f32, W [2048, 32, 32, 16] f32
          u_hat[b,r,j,o] = sum_i W[r,j,o,i] * in_caps[b,r,i]
          3 routing iterations:
            c = softmax_j(b_ij);  s[b,j,o] = sum_r c[r,j] u_hat[b,r,j,o]
            v = squash_o(s);      b_ij += (1/BS) sum_{b,o} u_hat v
          returns v[..., None]  -> [64, 32, 32, 1]

Strategy (per core, routes sharded 256/core; K = (r,i) = 4096 rows):
  * W shard resident in SBUF as fp16 Wt[(r,i), (j,o)]; u_hat never
    materialized.  Each iteration:
      pass 1: s = (c-scaled Wt) contracted with uT on PE (K=4096).
              One AllReduce of partial s [64, 1024] per iteration.
      pass 2: G = un.T @ v (PE);  tmp = Wt*G on DVE (2x mode, some
              groups on gpsimd);  o-reduce via one segmented
              tensor_reduce; i-reduce via a constant selector matmul
              accumulated in a persistent PSUM b_ij.
  * per 4-chunk group pipeline; the softmax/wc/pass1 consumption of
    group g is emitted one group late so the in-order PE queue never
    stalls on a wc that the DVE has not produced yet.
  * c is stored in duplicated-pair layout c2[p, chunk, j, 2] so the
    broadcast over o keeps the DVE multiply in 2x fp16 mode with no
    ACT-side expansion pass.
"""

import numpy as np

import concourse.bacc as bacc
import concourse.mybir as mybir
import concourse.tile as tile
from concourse.bass_utils import run_bass_kernel_spmd

BS, R, J, I, O = 64, 2048, 32, 16, 32
NUM_IT = 3
N_CORES = 8
R_LOC = R // N_CORES            # 256 routes per core
K_LOC = R_LOC * I               # 4096 contraction rows per core
NCHUNK = K_LOC // 128           # 32 chunks (8 routes x 16 i each)
GRP = 4                         # chunks per pipeline group
NGRP = NCHUNK // GRP            # 8 groups
JO = J * O                      # 1024
JH = JO // 2                    # 512 (one PSUM bank of f32)
F32 = mybir.dt.float32
FP16 = mybir.dt.float16
AX = mybir.AxisListType
ALU = mybir.AluOpType
ACTF = mybir.ActivationFunctionType



def _build_nc():
    nc = bacc.Bacc(trn_type="TRN2", target_bir_lowering=False, debug=False,
                   num_devices=N_CORES)
    wt = nc.dram_tensor("wt", [K_LOC, JO], FP16, kind="ExternalInput")
    ut = nc.dram_tensor("ut", [K_LOC, BS], FP16, kind="ExternalInput")
    un = nc.dram_tensor("un", [BS, K_LOC], FP16, kind="ExternalInput")
    sel = nc.dram_tensor("sel", [128, 128], FP16, kind="ExternalInput")
    vout = nc.dram_tensor("vout", [BS, JO], F32, kind="ExternalOutput")
    cc_wi = nc.dram_tensor("cc_wi", [1, 128], F32)
    cc_wo = nc.dram_tensor("cc_wo", [1, 128], F32, addr_space="Shared")
    cc_in = [nc.dram_tensor(f"cc_in{i}", [BS, JO], FP16)
             for i in range(NUM_IT)]
    cc_out = [nc.dram_tensor(f"cc_out{i}", [BS, JO], FP16,
                             addr_space="Shared")
              for i in range(NUM_IT)]
    rg = [list(range(N_CORES))]

    with tile.TileContext(nc) as tc:
        with (
            tc.tile_pool(name="big", bufs=1) as big,
            tc.tile_pool(name="wc", bufs=4) as wcp,
            tc.tile_pool(name="gsb", bufs=2) as gsbp,
            tc.tile_pool(name="wg", bufs=2) as wgp,
            tc.tile_pool(name="tr", bufs=2) as trp,
            tc.tile_pool(name="sm", bufs=2) as smp,
            tc.tile_pool(name="small", bufs=1) as small,
            tc.tile_pool(name="sps", bufs=1, space="PSUM") as spsp,
            tc.tile_pool(name="gps", bufs=2, space="PSUM") as gpsp,
            tc.tile_pool(name="bpsum", bufs=1, space="PSUM") as bpsum,
        ):
            # ---- resident tensors ----
            w_sb = big.tile([128, NCHUNK, JO], FP16)      # 64KB/part
            ut_sb = big.tile([128, NCHUNK, BS], FP16)
            un_sb = big.tile([BS, K_LOC], FP16)
            sel_sb = big.tile([128, 128], FP16)            # selector (1/64)
            e_sb = big.tile([128, NCHUNK, J], F32)        # exp(b) scratch
            c2_sb = big.tile([128, NCHUNK, J, 2], FP16)   # c_ij dup-pairs
            b_acc = bpsum.tile([128, NCHUNK, J], F32)     # persistent b_ij

            wt_v = wt.ap().rearrange("(c p) f -> c p f", p=128)
            ut_v = ut.ap().rearrange("(c p) f -> c p f", p=128)
            # interleave ut+w chunk loads over the 3 DMA queues so pass1
            # it0 can start as soon as the first chunks land; un/sel (only
            # needed by pass2, ~70us in) go behind the first few chunks
            _dengs = [nc.sync, nc.scalar, nc.gpsimd]
            for c in range(NCHUNK):
                eng = _dengs[c % 3]
                eng.dma_start(out=ut_sb[:, c, :], in_=ut_v[c])
                eng.dma_start(out=w_sb[:, c, :], in_=wt_v[c])
                if c == 8:
                    nc.sync.dma_start(out=un_sb, in_=un.ap())
                    nc.scalar.dma_start(out=sel_sb, in_=sel.ap())

            state = {}

            def emit_fill_load(n):
                """keep the PE clock ramped while DMA-bound: harmless
                matmuls into b_acc, which pass2-it0's selector start=True
                re-initializes before any real accumulation."""
                for _ in range(n):
                    nc.tensor.matmul(
                        out=b_acc[:, 0:GRP * 4, :], lhsT=sel_sb,
                        rhs=w_sb[:, 0, 0:JH],
                        start=True, stop=True, skip_group_check=True)

            def emit_fill_mid(n):
                """hold the PE pstate between real matmul bursts by
                streaming into the unused upper 64 partitions of s_ps
                (pass1 only writes rows 0-63); start=False never zeroes,
                so the live accumulation in rows 0-63 is untouched."""
                s_ps = state["s_ps"]
                for _ in range(n):
                    nc.tensor.matmul(
                        out=s_ps[BS:2 * BS, 0:JH], lhsT=ut_sb[:, 0, :],
                        rhs=w_sb[:, 0, 0:JH],
                        start=False, stop=False, skip_group_check=True)

            def emit_fill_ar(n):
                """bridge an AllReduce window: s_ps is free once copied
                to s_sb, so burn PE cycles there to hold the pstate."""
                s_ps = state["s_ps"]
                for _ in range(n):
                    nc.tensor.matmul(
                        out=s_ps[:BS, 0:JH], lhsT=ut_sb[:, 0, :],
                        rhs=w_sb[:, 0, 0:JH],
                        start=True, stop=True, skip_group_check=True)

            def emit_pass1_mm(it, g):
                """the pass-1 matmuls of group g (GRP chunks x 2 halves)."""
                if g == 0:
                    state["s_ps"] = spsp.tile([128, JO], F32, tag="s",
                                              name="s_ps")
                s_ps = state["s_ps"]
                c0 = GRP * g
                rhs = w_sb[:, c0:c0 + GRP, :] if it == 0 else state[("wc", g)]
                for ci in range(GRP):
                    for h in range(2):
                        nc.tensor.matmul(
                            out=s_ps[:BS, h * JH:(h + 1) * JH],
                            lhsT=ut_sb[:, c0 + ci, :],
                            rhs=rhs[:, ci, h * JH:(h + 1) * JH],
                            start=(c0 + ci == 0),
                            stop=(c0 + ci == NCHUNK - 1))

            def emit_ar_send(it):
                """s psum -> sbuf (gpsimd) -> DRAM (sync) -> AllReduce."""
                s_ps = state["s_ps"]
                s_sb = small.tile([BS, JO], FP16, tag="s_sb", name="s_sb")
                if it == 0:
                    nc.scalar.mul(s_sb, s_ps[:BS], 1.0 / J)
                else:
                    nc.scalar.copy(s_sb, s_ps[:BS])
                for q in range(4):
                    nc.sync.dma_start(
                        out=cc_in[it].ap()[:, q * 256:(q + 1) * 256],
                        in_=s_sb[:, q * 256:(q + 1) * 256])
                nc.gpsimd.collective_compute(
                    "AllReduce", ALU.add, replica_groups=rg,
                    ins=[cc_in[it].ap()], outs=[cc_out[it].ap()])

            def emit_ar_recv_squash(it):
                """DMA AR result back + squash; sets v_r halves."""
                s2 = small.tile([BS, J, O], FP16, tag=f"s2_{it % 2}",
                                name=f"s2_{it % 2}")
                s2f = s2.rearrange("p j o -> p (j o)")
                for q in range(4):
                    nc.sync.dma_start(
                        out=s2f[:, q * 256:(q + 1) * 256],
                        in_=cc_out[it].ap()[:, q * 256:(q + 1) * 256])
                ss = smp.tile([BS, J, O], F32, tag="ss", name="ss")
                nc.scalar.square(ss, s2)
                sq = smp.tile([BS, J], F32, tag="sq", name="sq")
                nc.vector.tensor_reduce(out=sq, in_=ss, axis=AX.X, op=ALU.add)
                rt = smp.tile([BS, J], F32, tag="rt", name="rt")
                nc.scalar.activation(rt, sq, ACTF.Sqrt)
                op1 = smp.tile([BS, J], F32, tag="op1", name="op1")
                nc.vector.tensor_scalar_add(op1, sq, 1.0)
                rden = smp.tile([BS, J], F32, tag="rden", name="rden")
                nc.vector.reciprocal(rden, op1)
                fac = smp.tile([BS, J], F32, tag="fac", name="fac")
                nc.vector.tensor_tensor(out=fac, in0=rt, in1=rden,
                                        op=ALU.mult)
                v_sb = small.tile([BS, J, O], F32, tag=f"v_{it % 2}",
                                  name=f"v_{it % 2}")
                nc.vector.tensor_tensor(
                    out=v_sb, in0=s2,
                    in1=fac.unsqueeze(2).broadcast_to([BS, J, O]),
                    op=ALU.mult)
                if it < NUM_IT - 1:
                    vf = v_sb.rearrange("p j o -> p (j o)")
                    for h in range(2):
                        v_r = small.tile([BS, JH], FP16,
                                         tag=f"vr{it % 2}_{h}",
                                         name=f"vr{it % 2}_{h}")
                        nc.scalar.copy(v_r, vf[:, h * JH:(h + 1) * JH])
                        state[("v_r", h)] = v_r
                return v_sb

            def emit_pass2_mm(it, g):
                """G matmuls + psum->sbuf copies for group g."""
                c0 = GRP * g
                g_sb = gsbp.tile([128, GRP, JO], FP16, name="g_sb")
                for ci in range(GRP):
                    g_ps = gpsp.tile([128, JO], F32, tag="g", name="g_ps")
                    for h in range(2):
                        nc.tensor.matmul(
                            out=g_ps[:, h * JH:(h + 1) * JH],
                            lhsT=un_sb[:, (c0 + ci) * 128:
                                       (c0 + ci + 1) * 128],
                            rhs=state[("v_r", h)],
                            start=True, stop=True)
                    nc.scalar.copy(g_sb[:, ci, :], g_ps)
                wg = wgp.tile([128, GRP, JO], FP16, name="wg",
                              bufs=4)
                nc.vector.tensor_tensor(out=wg, in0=w_sb[:, c0:c0 + GRP, :],
                                        in1=g_sb, op=ALU.mult)
                state[("wg", g)] = wg

            def emit_bup(it, g):
                """o-reduce 32->4 as a tensor_tensor tree: TT adds run in
                2x fp16 mode while tensor_reduce is only 1x."""
                wg = state.pop(("wg", g))
                wgv = wg.rearrange("p c (j o) -> p (c j) o", o=O)
                t16 = trp.tile([128, GRP * J, 16], FP16, name="t16")
                nc.vector.tensor_tensor(out=t16, in0=wgv[:, :, 0:16],
                                        in1=wgv[:, :, 16:32], op=ALU.add)
                t8 = trp.tile([128, GRP * J, 8], FP16, name="t8")
                nc.vector.tensor_tensor(out=t8, in0=t16[:, :, 0:8],
                                        in1=t16[:, :, 8:16], op=ALU.add)
                t4 = trp.tile([128, GRP * J, 4], FP16, name="t4")
                nc.vector.tensor_tensor(out=t4, in0=t8[:, :, 0:4],
                                        in1=t8[:, :, 4:8], op=ALU.add)
                state[("bup", g)] = t4

            def emit_bsel_softmax_wc(it, g):
                """selector i-reduce, softmax refresh and wc for group g."""
                c0 = GRP * g
                # b_acc spans 2 PSUM banks; start/stop once per bank
                t4 = state.pop(("bup", g))
                for oo in range(4):
                    nc.tensor.matmul(
                        out=b_acc[:, c0:c0 + GRP, :], lhsT=sel_sb,
                        rhs=t4[:, :, oo],
                        start=(it == 0 and g in (0, NGRP // 2) and oo == 0),
                        stop=(it == NUM_IT - 2
                              and g in (NGRP // 2 - 1, NGRP - 1)
                              and oo == 3),
                        skip_group_check=True)
                nc.scalar.activation(e_sb[:, c0:c0 + GRP, :],
                                     b_acc[:, c0:c0 + GRP, :], ACTF.Exp)
                esum = smp.tile([128, GRP], F32, tag="esum", name="esum")
                nc.vector.tensor_reduce(
                    out=esum, in_=e_sb[:, c0:c0 + GRP, :],
                    axis=AX.X, op=ALU.add)
                erec = smp.tile([128, GRP], F32, tag="erec", name="erec")
                nc.vector.reciprocal(erec, esum)
                nc.vector.tensor_tensor(
                    out=c2_sb[:, c0:c0 + GRP, :, :],
                    in0=e_sb[:, c0:c0 + GRP, :].unsqueeze(3)
                        .broadcast_to([128, GRP, J, 2]),
                    in1=erec.unsqueeze(2).unsqueeze(3)
                        .broadcast_to([128, GRP, J, 2]),
                    op=ALU.mult)
                # c-scaled W for pass1 of it+1
                wc_t = wcp.tile([128, GRP, JO], FP16, name="wc_t")
                nc.vector.tensor_tensor(
                    out=wc_t.rearrange("p c (j o2 t) -> p c j o2 t",
                                       o2=O // 2, t=2),
                    in0=w_sb[:, c0:c0 + GRP, :].rearrange(
                        "p c (j o2 t) -> p c j o2 t", o2=O // 2, t=2),
                    in1=c2_sb[:, c0:c0 + GRP, :, :].unsqueeze(3)
                        .broadcast_to([128, GRP, J, O // 2, 2]),
                    op=ALU.mult)
                state[("wc", g)] = wc_t

            # ---- main schedule ----
            for g in range(NGRP):
                emit_pass1_mm(0, g)
                if g < NGRP - 1:
                    emit_fill_load(6)
            emit_ar_send(0)
            emit_fill_ar(56)

            v_sb = None
            for it in range(NUM_IT):
                v_sb = emit_ar_recv_squash(it)
                if it < NUM_IT - 1:
                    # delay-2 slot pipeline: slot g runs pass2(g) on
                    # PE/ACT/DVE heads, the softmax/wc chain of g-1 mid
                    # slot, and pass1(it+1) of g-2 -- so no engine queue
                    # ever head-of-line blocks on work from this slot
                    for g in range(NGRP + 2):
                        if g < NGRP:
                            emit_pass2_mm(it, g)
                        if 1 <= g <= NGRP:
                            emit_bup(it, g - 1)
                            emit_bsel_softmax_wc(it, g - 1)
                        if g >= 2:
                            emit_pass1_mm(it + 1, g - 2)
                            emit_fill_mid(4)
                    emit_ar_send(it + 1)
                    emit_fill_ar(40)

            v_flat_out = v_sb.rearrange("p j o -> p (j o)")
            for q in range(4):
                nc.sync.dma_start(out=vout.ap()[:, q * 256:(q + 1) * 256],
                                  in_=v_flat_out[:, q * 256:(q + 1) * 256])
    nc.finalize()
    return nc


_NC_CACHE = {}
TRACE = False
TRACE_CORES = None


def _get_nc():
    if "nc" not in _NC_CACHE:
        _NC_CACHE["nc"] = _build_nc()
    return _NC_CACHE["nc"]


def _make_sel():
    sel = np.zeros((128, 128), np.float32)
    for p in range(128):
        m0 = (p // 16) * 16
        sel[p, m0:m0 + 16] = 1.0 / BS
    return sel


def kernel(**inputs):
    in_caps = np.ascontiguousarray(inputs["in_caps"], dtype=np.float32)
    W = np.ascontiguousarray(inputs["W"], dtype=np.float32)
    assert in_caps.shape == (BS, R, I) and W.shape == (R, J, O, I)

    bf = np.float16
    Wt = np.ascontiguousarray(
        W.transpose(0, 3, 1, 2).reshape(R * I, J * O).astype(bf))
    uT = np.ascontiguousarray(
        in_caps.transpose(1, 2, 0).reshape(R * I, BS).astype(bf))
    un = np.ascontiguousarray(in_caps.reshape(BS, R * I).astype(bf))
    sel = _make_sel().astype(np.float16)

    in_maps = []
    for k in range(N_CORES):
        rows = slice(k * K_LOC, (k + 1) * K_LOC)
        in_maps.append({
            "wt": np.ascontiguousarray(Wt[rows]),
            "ut": np.ascontiguousarray(uT[rows]),
            "un": np.ascontiguousarray(un[:, rows]),
            "sel": sel,
        })

    nc = _get_nc()
    res = run_bass_kernel_spmd(nc, in_maps, core_ids=list(range(N_CORES)),
                               trace=TRACE, trace_cores=TRACE_CORES)
    _NC_CACHE["last_result"] = res
    v = np.asarray(res.results[0]["vout"], dtype=np.float32)
    return v.reshape(BS, J, O, 1)


if __name__ == "__main__":
    rng = np.random.default_rng(0)
    ins = {
        "in_caps": rng.standard_normal((BS, R, I), dtype=np.float32),
        "W": rng.standard_normal((R, J, O, I), dtype=np.float32),
    }
    out = kernel(**ins)
    print(out.shape, out.dtype, np.abs(out).mean())


# revision 27
# speedup vs baseline: 1.0365x; 1.0365x over previous
"""DigitCaps dynamic-routing kernel for Trainium2, 8 NeuronCores (SPMD).

Problem:  in_caps [64, 2048, 16] f32, W [2048, 32, 32, 16] f32
          u_hat[b,r,j,o] = sum_i W[r,j,o,i] * in_caps[b,r,i]
          3 routing iterations:
            c = softmax_j(b_ij);  s[b,j,o] = sum_r c[r,j] u_hat[b,r,j,o]
            v = squash_o(s);      b_ij += (1/BS) sum_{b,o} u_hat v
          returns v[..., None]  -> [64, 32, 32, 1]

Strategy (per core, routes sharded 256/core; K = (r,i) = 4096 rows):
  * W shard resident in SBUF as fp16 Wt[(r,i), (j,o)]; u_hat never
    materialized.  Each iteration:
      pass 1: s = (c-scaled Wt) contracted with uT on PE (K=4096).
              One AllReduce of partial s [64, 1024] per iteration.
      pass 2: G = un.T @ v (PE);  tmp = Wt*G on DVE (2x mode, some
              groups on gpsimd);  o-reduce via one segmented
              tensor_reduce; i-reduce via a constant selector matmul
              accumulated in a persistent PSUM b_ij.
  * per 4-chunk group pipeline; the softmax/wc/pass1 consumption of
    group g is emitted one group late so the in-order PE queue never
    stalls on a wc that the DVE has not produced yet.
  * c is stored in duplicated-pair layout c2[p, chunk, j, 2] so the
    broadcast over o keeps the DVE multiply in 2x fp16 mode with no
    ACT-side expansion pass.
"""

import numpy as np

import concourse.bacc as bacc
import concourse.mybir as mybir
import concourse.tile as tile
from concourse.bass_utils import run_bass_kernel_spmd

BS, R, J, I, O = 64, 2048, 32, 16, 32
NUM_IT = 3
N_CORES = 8
R_LOC = R // N_CORES            # 256 routes per core
K_LOC = R_LOC * I               # 4096 contraction rows per core
NCHUNK = K_LOC // 128           # 32 chunks (8 routes x 16 i each)
GRP = 4                         # chunks per pipeline group
NGRP = NCHUNK // GRP            # 8 groups
JO = J * O                      # 1024
JH = JO // 2                    # 512 (one PSUM bank of f32)
F32 = mybir.dt.float32
FP16 = mybir.dt.float16
AX = mybir.AxisListType
ALU = mybir.AluOpType
ACTF = mybir.ActivationFunctionType



def _build_nc():
    nc = bacc.Bacc(trn_type="TRN2", target_bir_lowering=False, debug=False,
                   num_devices=N_CORES)
    wt = nc.dram_tensor("wt", [K_LOC, JO], FP16, kind="ExternalInput")
    ut = nc.dram_tensor("ut", [K_LOC, BS], FP16, kind="ExternalInput")
    un = nc.dram_tensor("un", [BS, K_LOC], FP16, kind="ExternalInput")
    sel = nc.dram_tensor("sel", [128, 128], FP16, kind="ExternalInput")
    vout = nc.dram_tensor("vout", [BS, JO], F32, kind="ExternalOutput")
    cc_wi = nc.dram_tensor("cc_wi", [1, 128], F32)
    cc_wo = nc.dram_tensor("cc_wo", [1, 128], F32, addr_space="Shared")
    cc_in = [nc.dram_tensor(f"cc_in{i}", [BS, JO], FP16)
             for i in range(NUM_IT)]
    cc_out = [nc.dram_tensor(f"cc_out{i}", [BS, JO], FP16,
                             addr_space="Shared")
              for i in range(NUM_IT)]
    rg = [list(range(N_CORES))]

    with tile.TileContext(nc) as tc:
        with (
            tc.tile_pool(name="big", bufs=1) as big,
            tc.tile_pool(name="wc", bufs=4) as wcp,
            tc.tile_pool(name="gsb", bufs=2) as gsbp,
            tc.tile_pool(name="wg", bufs=2) as wgp,
            tc.tile_pool(name="tr", bufs=2) as trp,
            tc.tile_pool(name="sm", bufs=2) as smp,
            tc.tile_pool(name="small", bufs=1) as small,
            tc.tile_pool(name="sps", bufs=1, space="PSUM") as spsp,
            tc.tile_pool(name="gps", bufs=2, space="PSUM") as gpsp,
            tc.tile_pool(name="bpsum", bufs=1, space="PSUM") as bpsum,
        ):
            # ---- resident tensors ----
            w_sb = big.tile([128, NCHUNK, JO], FP16)      # 64KB/part
            ut_sb = big.tile([128, NCHUNK, BS], FP16)
            un_sb = big.tile([BS, K_LOC], FP16)
            sel_sb = big.tile([128, 128], FP16)            # selector (1/64)
            e_sb = big.tile([128, NCHUNK, J], F32)        # exp(b) scratch
            c2_sb = big.tile([128, NCHUNK, J, 2], FP16)   # c_ij dup-pairs
            b_acc = bpsum.tile([128, NCHUNK, J], F32)     # persistent b_ij

            wt_v = wt.ap().rearrange("(c p) f -> c p f", p=128)
            ut_v = ut.ap().rearrange("(c p) f -> c p f", p=128)
            # interleave ut+w chunk loads over the 3 DMA queues so pass1
            # it0 can start as soon as the first chunks land; un/sel (only
            # needed by pass2, ~70us in) go behind the first few chunks
            _dengs = [nc.sync, nc.scalar, nc.gpsimd]
            for c in range(NCHUNK):
                eng = _dengs[c % 3]
                eng.dma_start(out=ut_sb[:, c, :], in_=ut_v[c])
                eng.dma_start(out=w_sb[:, c, :], in_=wt_v[c])
                if c == 8:
                    nc.sync.dma_start(out=un_sb, in_=un.ap())
                    nc.scalar.dma_start(out=sel_sb, in_=sel.ap())

            state = {}

            def emit_fill_load(n):
                """keep the PE clock ramped while DMA-bound: harmless
                matmuls into b_acc, which pass2-it0's selector start=True
                re-initializes before any real accumulation."""
                for _ in range(n):
                    nc.tensor.matmul(
                        out=b_acc[:, 0:GRP * 4, :], lhsT=sel_sb,
                        rhs=w_sb[:, 0, 0:JH],
                        start=True, stop=True, skip_group_check=True)

            def emit_fill_mid(n):
                """hold the PE pstate between real matmul bursts by
                streaming into the unused upper 64 partitions of s_ps
                (pass1 only writes rows 0-63); start=False never zeroes,
                so the live accumulation in rows 0-63 is untouched."""
                s_ps = state["s_ps"]
                for _ in range(n):
                    nc.tensor.matmul(
                        out=s_ps[BS:2 * BS, 0:JH], lhsT=ut_sb[:, 0, :],
                        rhs=w_sb[:, 0, 0:JH],
                        start=False, stop=False, skip_group_check=True)

            def emit_fill_ar(n):
                """bridge an AllReduce window: s_ps is free once copied
                to s_sb, so burn PE cycles there to hold the pstate."""
                s_ps = state["s_ps"]
                for _ in range(n):
                    nc.tensor.matmul(
                        out=s_ps[:BS, 0:JH], lhsT=ut_sb[:, 0, :],
                        rhs=w_sb[:, 0, 0:JH],
                        start=True, stop=True, skip_group_check=True)

            def emit_pass1_mm(it, g):
                """the pass-1 matmuls of group g (GRP chunks x 2 halves)."""
                if g == 0:
                    state["s_ps"] = spsp.tile([128, JO], F32, tag="s",
                                              name="s_ps")
                s_ps = state["s_ps"]
                c0 = GRP * g
                rhs = w_sb[:, c0:c0 + GRP, :] if it == 0 else state[("wc", g)]
                for ci in range(GRP):
                    for h in range(2):
                        nc.tensor.matmul(
                            out=s_ps[:BS, h * JH:(h + 1) * JH],
                            lhsT=ut_sb[:, c0 + ci, :],
                            rhs=rhs[:, ci, h * JH:(h + 1) * JH],
                            start=(c0 + ci == 0),
                            stop=(c0 + ci == NCHUNK - 1))

            def emit_ar_send(it):
                """s psum -> sbuf (gpsimd) -> DRAM (sync) -> AllReduce."""
                s_ps = state["s_ps"]
                s_sb = small.tile([BS, JO], FP16, tag="s_sb", name="s_sb")
                if it == 0:
                    nc.scalar.mul(s_sb, s_ps[:BS], 1.0 / J)
                else:
                    nc.scalar.copy(s_sb, s_ps[:BS])
                for q in range(4):
                    nc.sync.dma_start(
                        out=cc_in[it].ap()[:, q * 256:(q + 1) * 256],
                        in_=s_sb[:, q * 256:(q + 1) * 256])
                nc.gpsimd.collective_compute(
                    "AllReduce", ALU.add, replica_groups=rg,
                    ins=[cc_in[it].ap()], outs=[cc_out[it].ap()])

            def emit_ar_recv_squash(it):
                """DMA AR result back + squash; sets v_r halves."""
                s2 = small.tile([BS, J, O], FP16, tag=f"s2_{it % 2}",
                                name=f"s2_{it % 2}")
                s2f = s2.rearrange("p j o -> p (j o)")
                for q in range(4):
                    nc.sync.dma_start(
                        out=s2f[:, q * 256:(q + 1) * 256],
                        in_=cc_out[it].ap()[:, q * 256:(q + 1) * 256])
                ss = smp.tile([BS, J, O], F32, tag="ss", name="ss")
                nc.scalar.square(ss, s2)
                sq = smp.tile([BS, J], F32, tag="sq", name="sq")
                nc.vector.tensor_reduce(out=sq, in_=ss, axis=AX.X, op=ALU.add)
                rt = smp.tile([BS, J], F32, tag="rt", name="rt")
                nc.scalar.activation(rt, sq, ACTF.Sqrt)
                op1 = smp.tile([BS, J], F32, tag="op1", name="op1")
                nc.vector.tensor_scalar_add(op1, sq, 1.0)
                rden = smp.tile([BS, J], F32, tag="rden", name="rden")
                nc.vector.reciprocal(rden, op1)
                fac = smp.tile([BS, J], F32, tag="fac", name="fac")
                nc.vector.tensor_tensor(out=fac, in0=rt, in1=rden,
                                        op=ALU.mult)
                v_sb = small.tile([BS, J, O], F32, tag=f"v_{it % 2}",
                                  name=f"v_{it % 2}")
                nc.vector.tensor_tensor(
                    out=v_sb, in0=s2,
                    in1=fac.unsqueeze(2).broadcast_to([BS, J, O]),
                    op=ALU.mult)
                if it < NUM_IT - 1:
                    vf = v_sb.rearrange("p j o -> p (j o)")
                    for h in range(2):
                        v_r = small.tile([BS, JH], FP16,
                                         tag=f"vr{it % 2}_{h}",
                                         name=f"vr{it % 2}_{h}")
                        nc.scalar.copy(v_r, vf[:, h * JH:(h + 1) * JH])
                        state[("v_r", h)] = v_r
                return v_sb

            def emit_pass2_mm(it, g):
                """G matmuls + psum->sbuf copies for group g."""
                c0 = GRP * g
                g_sb = gsbp.tile([128, GRP, JO], FP16, name="g_sb")
                for ci in range(GRP):
                    g_ps = gpsp.tile([128, JO], F32, tag="g", name="g_ps")
                    for h in range(2):
                        nc.tensor.matmul(
                            out=g_ps[:, h * JH:(h + 1) * JH],
                            lhsT=un_sb[:, (c0 + ci) * 128:
                                       (c0 + ci + 1) * 128],
                            rhs=state[("v_r", h)],
                            start=True, stop=True)
                    nc.scalar.copy(g_sb[:, ci, :], g_ps)
                wg = wgp.tile([128, GRP, JO], FP16, name="wg",
                              bufs=4)
                nc.vector.tensor_tensor(out=wg, in0=w_sb[:, c0:c0 + GRP, :],
                                        in1=g_sb, op=ALU.mult)
                state[("wg", g)] = wg

            def emit_bup(it, g):
                """o-reduce 32->4 as a tensor_tensor tree: TT adds run in
                2x fp16 mode while tensor_reduce is only 1x."""
                wg = state.pop(("wg", g))
                wgv = wg.rearrange("p c (j o) -> p (c j) o", o=O)
                t16 = trp.tile([128, GRP * J, 16], FP16, name="t16")
                nc.vector.tensor_tensor(out=t16, in0=wgv[:, :, 0:16],
                                        in1=wgv[:, :, 16:32], op=ALU.add)
                t8 = trp.tile([128, GRP * J, 8], FP16, name="t8")
                nc.vector.tensor_tensor(out=t8, in0=t16[:, :, 0:8],
                                        in1=t16[:, :, 8:16], op=ALU.add)
                t4 = trp.tile([128, GRP * J, 4], FP16, name="t4")
                nc.vector.tensor_tensor(out=t4, in0=t8[:, :, 0:4],
                                        in1=t8[:, :, 4:8], op=ALU.add)
                state[("bup", g)] = t4

            def emit_bsel_softmax_wc(it, g):
                """selector i-reduce, softmax refresh and wc for group g."""
                c0 = GRP * g
                # b_acc spans 2 PSUM banks; start/stop once per bank
                t4 = state.pop(("bup", g))
                for oo in range(4):
                    nc.tensor.matmul(
                        out=b_acc[:, c0:c0 + GRP, :], lhsT=sel_sb,
                        rhs=t4[:, :, oo],
                        start=(it == 0 and g in (0, NGRP // 2) and oo == 0),
                        stop=(it == NUM_IT - 2
                              and g in (NGRP // 2 - 1, NGRP - 1)
                              and oo == 3),
                        skip_group_check=True)
                nc.scalar.activation(e_sb[:, c0:c0 + GRP, :],
                                     b_acc[:, c0:c0 + GRP, :], ACTF.Exp)
                esum = smp.tile([128, GRP], F32, tag="esum", name="esum")
                nc.vector.tensor_reduce(
                    out=esum, in_=e_sb[:, c0:c0 + GRP, :],
                    axis=AX.X, op=ALU.add)
                erec = smp.tile([128, GRP], F32, tag="erec", name="erec")
                nc.vector.reciprocal(erec, esum)
                nc.vector.tensor_tensor(
                    out=c2_sb[:, c0:c0 + GRP, :, :],
                    in0=e_sb[:, c0:c0 + GRP, :].unsqueeze(3)
                        .broadcast_to([128, GRP, J, 2]),
                    in1=erec.unsqueeze(2).unsqueeze(3)
                        .broadcast_to([128, GRP, J, 2]),
                    op=ALU.mult)
                # c-scaled W for pass1 of it+1
                wc_t = wcp.tile([128, GRP, JO], FP16, name="wc_t")
                nc.vector.tensor_tensor(
                    out=wc_t.rearrange("p c (j o2 t) -> p c j o2 t",
                                       o2=O // 2, t=2),
                    in0=w_sb[:, c0:c0 + GRP, :].rearrange(
                        "p c (j o2 t) -> p c j o2 t", o2=O // 2, t=2),
                    in1=c2_sb[:, c0:c0 + GRP, :, :].unsqueeze(3)
                        .broadcast_to([128, GRP, J, O // 2, 2]),
                    op=ALU.mult)
                state[("wc", g)] = wc_t

            # ---- main schedule ----
            for g in range(NGRP):
                emit_pass1_mm(0, g)
                if g < NGRP - 1:
                    emit_fill_load(6)
            emit_ar_send(0)
            emit_fill_ar(56)

            v_sb = None
            for it in range(NUM_IT):
                v_sb = emit_ar_recv_squash(it)
                if it < NUM_IT - 1:
                    # delay-2 slot pipeline: slot g runs pass2(g) on
                    # PE/ACT/DVE heads, the softmax/wc chain of g-1 mid
                    # slot, and pass1(it+1) of g-2 -- so no engine queue
                    # ever head-of-line blocks on work from this slot
                    for g in range(NGRP + 2):
                        if g < NGRP:
                            emit_pass2_mm(it, g)
                        if 1 <= g <= NGRP:
                            emit_bup(it, g - 1)
                            emit_bsel_softmax_wc(it, g - 1)
                        if g >= 2:
                            emit_pass1_mm(it + 1, g - 2)
                            emit_fill_mid(4)
                    emit_ar_send(it + 1)
                    if it + 1 < NUM_IT - 1:
                        emit_fill_ar(40)

            v_flat_out = v_sb.rearrange("p j o -> p (j o)")
            for q in range(4):
                nc.sync.dma_start(out=vout.ap()[:, q * 256:(q + 1) * 256],
                                  in_=v_flat_out[:, q * 256:(q + 1) * 256])
    nc.finalize()
    return nc


_NC_CACHE = {}
TRACE = False
TRACE_CORES = None


def _get_nc():
    if "nc" not in _NC_CACHE:
        _NC_CACHE["nc"] = _build_nc()
    return _NC_CACHE["nc"]


def _make_sel():
    sel = np.zeros((128, 128), np.float32)
    for p in range(128):
        m0 = (p // 16) * 16
        sel[p, m0:m0 + 16] = 1.0 / BS
    return sel


def kernel(**inputs):
    in_caps = np.ascontiguousarray(inputs["in_caps"], dtype=np.float32)
    W = np.ascontiguousarray(inputs["W"], dtype=np.float32)
    assert in_caps.shape == (BS, R, I) and W.shape == (R, J, O, I)

    bf = np.float16
    Wt = np.ascontiguousarray(
        W.transpose(0, 3, 1, 2).reshape(R * I, J * O).astype(bf))
    uT = np.ascontiguousarray(
        in_caps.transpose(1, 2, 0).reshape(R * I, BS).astype(bf))
    un = np.ascontiguousarray(in_caps.reshape(BS, R * I).astype(bf))
    sel = _make_sel().astype(np.float16)

    in_maps = []
    for k in range(N_CORES):
        rows = slice(k * K_LOC, (k + 1) * K_LOC)
        in_maps.append({
            "wt": np.ascontiguousarray(Wt[rows]),
            "ut": np.ascontiguousarray(uT[rows]),
            "un": np.ascontiguousarray(un[:, rows]),
            "sel": sel,
        })

    nc = _get_nc()
    res = run_bass_kernel_spmd(nc, in_maps, core_ids=list(range(N_CORES)),
                               trace=TRACE, trace_cores=TRACE_CORES)
    _NC_CACHE["last_result"] = res
    v = np.asarray(res.results[0]["vout"], dtype=np.float32)
    return v.reshape(BS, J, O, 1)


if __name__ == "__main__":
    rng = np.random.default_rng(0)
    ins = {
        "in_caps": rng.standard_normal((BS, R, I), dtype=np.float32),
        "W": rng.standard_normal((R, J, O, I), dtype=np.float32),
    }
    out = kernel(**ins)
    print(out.shape, out.dtype, np.abs(out).mean())
